# revision 32
# baseline (speedup 1.0000x reference)
"""Multi-head attention (RoPE + causal) on 8 TRN2 NeuronCores.

Sharding: tensor-parallel over heads. Core c owns heads {2c, 2c+1} for
both batches: Wqkv column-sharded by head, Wproj row-sharded; each core
produces a partial [B*S, D] output which the host sums.

Per-core pipeline (all matmuls bf16, fp32 PSUM accumulation):
  - x is pre-transposed/cast on host to xT[b] = [D, S] bf16 so the
    contraction dim (d) lands on SBUF partitions.
  - Q^T/K^T computed head-major ([hd, tok]); RoPE applied in that
    layout via partition-shifted DVE ops with host-prepared
    transposed (and sign-folded) sin/cos tables.
  - V computed token-major ([tok, hd]) from the same xT tiles.
  - Scores computed TRANSPOSED: S^T[k, q] = K_tile @ Q^T so softmax'a
    exp runs on ACT straight out of PSUM, probs stay [k, q] = P^T,
    which is exactly the moving operand PV needs -> no on-chip
    transposes anywhere. Row sums via a ones-vector matmul; causal
    handled structurally (upper tiles never computed) + 4 static
    diagonal masks; no max-subtraction (scores are O(1) by
    construction so exp cannot overflow).
  - att^T[hd, q] accumulates in PSUM, normalized by 1/sum broadcast
    (GPSIMD partition_broadcast), and is directly the stationary
    operand for the output projection.
"""

import os
import sys

for _p in ("/opt/trn_rl_repo", "/root/.axon_site/_ro/trn_rl_repo"):
    if os.path.isdir(_p) and _p not in sys.path:
        sys.path.append(_p)

import numpy as np
import ml_dtypes

import concourse.bass as bass
import concourse.mybir as mybir
import concourse.tile as tile
from concourse import bacc
from concourse.bass_utils import run_bass_kernel_spmd

B, S, D, H, HD = 2, 2048, 2048, 16, 128
NCORES = 8
HPC = H // NCORES            # heads per core = 2
QKC = 2 * HPC * HD           # q+k cols per core = 512
VC = HPC * HD                # v cols per core = 256
DC = D // 128                # d chunks = 16
TB = 4                       # token blocks of 512
BLK = S // TB                # 512
SCALE = 1.0 / float(np.sqrt(HD))

bf16 = mybir.dt.bfloat16
f32 = mybir.dt.float32

_cached = {}


def _build():
    if "nc" in _cached:
        return _cached["nc"]
    from contextlib import ExitStack

    nc = bacc.Bacc("TRN2", target_bir_lowering=False, debug=False,
                   num_devices=NCORES)

    xt = nc.dram_tensor("xt", [B, D, S], bf16, kind="ExternalInput")
    wqk = nc.dram_tensor("wqk", [D, QKC], bf16, kind="ExternalInput")
    wv = nc.dram_tensor("wv", [D, VC], bf16, kind="ExternalInput")
    wp = nc.dram_tensor("wp", [VC, D], bf16, kind="ExternalInput")
    cost = nc.dram_tensor("cost", [HD, S], bf16, kind="ExternalInput")
    sints = nc.dram_tensor("sints", [HD, S], bf16, kind="ExternalInput")
    dmask = nc.dram_tensor("dmask", [4, 128, BLK], bf16, kind="ExternalInput")
    out = nc.dram_tensor("out", [B * S, D], f32, kind="ExternalOutput")

    with tile.TileContext(nc) as tc, ExitStack() as ctx:
        cpool = ctx.enter_context(tc.tile_pool(name="consts", bufs=1))
        xt_pool = ctx.enter_context(tc.tile_pool(name="xt", bufs=2))
        qk_pool = ctx.enter_context(tc.tile_pool(name="qk", bufs=2))
        v_pool = ctx.enter_context(tc.tile_pool(name="v", bufs=2))
        att_pool = ctx.enter_context(tc.tile_pool(name="att", bufs=2))
        rope_pool = ctx.enter_context(tc.tile_pool(name="rope", bufs=2))
        probs_pool = ctx.enter_context(tc.tile_pool(name="probs", bufs=6))
        norm_pool = ctx.enter_context(tc.tile_pool(name="norm", bufs=2))
        out_pool = ctx.enter_context(tc.tile_pool(name="out", bufs=4))
        # 8 PSUM banks total: work 2x1 (QK/V accum in qkv phase, PV
        # accumulators in attention - phase-disjoint), 2-bank pairs 2x2
        # (attention scores + proj output), sum 2x1. Double-buffered
        # everywhere so consecutive q-blocks pipeline with no PE stall.
        ps_work = ctx.enter_context(tc.tile_pool(name="pswork", bufs=3, space="PSUM"))
        ps_att = ctx.enter_context(tc.tile_pool(name="psatt", bufs=2, space="PSUM"))
        ps_sum = ctx.enter_context(tc.tile_pool(name="pssum", bufs=1, space="PSUM"))

        # DMA issue order matters for startup: wqk + first xt block feed the
        # first matmul group; everything else can land later.
        wqk_sb = cpool.tile([128, DC, QKC], bf16)
        wqk_r = wqk.rearrange("(c p) n -> p c n", p=128)
        nc.sync.dma_start(wqk_sb[:, 0:DC // 2, :], wqk_r[:, 0:DC // 2, :])
        xt0 = xt_pool.tile([128, DC, BLK], bf16, tag="xtb")
        xt0_r = xt[0].rearrange("(c p) s -> p c s", p=128)[:, :, 0:BLK]
        nc.sync.dma_start(xt0[:, 0:DC // 2, :], xt0_r[:, 0:DC // 2, :])
        nc.sync.dma_start(wqk_sb[:, DC // 2:DC, :], wqk_r[:, DC // 2:DC, :])
        nc.sync.dma_start(xt0[:, DC // 2:DC, :], xt0_r[:, DC // 2:DC, :])
        cost_sb = cpool.tile([HD, S], bf16)
        nc.sync.dma_start(cost_sb[:], cost[:, :])
        sints_sb = cpool.tile([HD, S], bf16)
        nc.sync.dma_start(sints_sb[:], sints[:, :])
        # prefetch the second token block ahead of the late-needed consts
        xt1 = xt_pool.tile([128, DC, BLK], bf16, tag="xtb")
        nc.sync.dma_start(
            xt1[:], xt[0].rearrange("(c p) s -> p c s", p=128)[:, :, BLK:2 * BLK])
        wv_sb = cpool.tile([128, DC, VC], bf16)
        nc.sync.dma_start(wv_sb[:], wv.rearrange("(c p) n -> p c n", p=128))
        dmask_sb = cpool.tile([128, 4, BLK], bf16)
        nc.sync.dma_start(dmask_sb[:], dmask.rearrange("d p n -> p d n"))
        wp_sb = cpool.tile([128, HPC, D], bf16)
        nc.sync.dma_start(wp_sb[:], wp.rearrange("(c p) n -> p c n", p=128))
        ones_sb = cpool.tile([128, 1], bf16)
        nc.vector.memset(ones_sb[:], 1.0)

        # PE warm-up during the initial DMA window: keeps the HAM activity
        # monitor busy so the first real matmuls run at full clock.
        warm_sb = cpool.tile([128, BLK], bf16)
        nc.vector.memset(warm_sb[:], 1.0)
        warm_ps = ps_work.tile([128, BLK], f32, tag="work")
        for _ in range(40):
            nc.tensor.matmul(warm_ps[:], warm_sb[:, 0:128], warm_sb[:],
                             start=True, stop=True)
        nc.vector.tensor_copy(warm_sb[:], warm_ps[:])

        for b in range(B):
            qk_b = qk_pool.tile([128, 2 * HPC, S], bf16, tag="qkb")
            v_b = v_pool.tile([128, 4 * TB, VC], bf16, tag="vb")
            att_b = att_pool.tile([128, HPC, S], bf16, tag="attb")

            # ---- QKV projection for batch b ----
            for tb in range(TB):
                if b == 0 and tb == 0:
                    xt_blk = xt0
                elif b == 0 and tb == 1:
                    xt_blk = xt1
                elif b == 1 and tb == 0:
                    xt_blk = xt_next
                else:
                    xt_blk = xt_pool.tile([128, DC, BLK], bf16, tag="xtb")
                    nc.sync.dma_start(
                        xt_blk[:],
                        xt[b].rearrange("(c p) s -> p c s", p=128)[
                            :, :, bass.ts(tb, BLK)],
                    )
                # Q^T / K^T (head-major) + RoPE
                for ct in range(2 * HPC):
                    ps = ps_work.tile([128, BLK], f32, tag="work")
                    for c in range(DC):
                        nc.tensor.matmul(
                            ps[:], wqk_sb[:, c, bass.ts(ct, 128)],
                            xt_blk[:, c, :],
                            start=(c == 0), stop=(c == DC - 1))
                    cs = cost_sb[:, bass.ts(tb, BLK)]
                    sn = sints_sb[:, bass.ts(tb, BLK)]
                    t1 = rope_pool.tile([128, BLK], f32, tag="t1")
                    t2 = rope_pool.tile([128, BLK], f32, tag="t2")
                    # t1 = rot_half(ps) * sin (sign folded into table)
                    nc.vector.tensor_mul(t1[0:64, :], ps[64:128, :], sn[0:64, :])
                    nc.vector.tensor_mul(t1[64:128, :], ps[0:64, :], sn[64:128, :])
                    nc.vector.tensor_mul(t2[:], ps[:], cs)
                    nc.vector.tensor_add(
                        qk_b[:, ct, bass.ts(tb, BLK)], t1[:], t2[:])
                # V (token-major)
                for ts4 in range(4):
                    psv = ps_work.tile([128, VC], f32, tag="work")
                    for c in range(DC):
                        nc.tensor.matmul(
                            psv[:], xt_blk[:, c, bass.ts(ts4, 128)],
                            wv_sb[:, c, :],
                            start=(c == 0), stop=(c == DC - 1))
                    nc.vector.tensor_copy(v_b[:, tb * 4 + ts4, :], psv[:])

            if b == 0:
                # prefetch batch 1's first token block now: the xt slot is
                # free through b0's attention/proj, and issuing the DMA here
                # keeps it ahead of proj0's out-DMAs in the sync queue.
                xt_next = xt_pool.tile([128, DC, BLK], bf16, tag="xtb")
                nc.sync.dma_start(
                    xt_next[:],
                    xt[1].rearrange("(c p) s -> p c s", p=128)[:, :, 0:BLK])

            # ---- causal attention, heads h = 0..HPC-1 ----
            for h in range(HPC):
                qh = qk_b[:, h, :]
                kh = qk_b[:, HPC + h, :]
                for qb in range(TB):
                    nk = 4 * (qb + 1)
                    np_ = nk // 2
                    po = ps_work.tile([128, BLK], f32, tag="work")
                    psm = ps_sum.tile([1, BLK], f32, tag="pssum")
                    ktile = []

                    def pv(kt):
                        ap, c0 = ktile[kt]
                        nc.tensor.matmul(
                            po[:, c0:BLK], v_b[:, kt, bass.ts(h, HD)], ap,
                            start=(kt == 0), stop=(kt == nk - 1))

                    def ones_mm(kt):
                        ap, c0 = ktile[kt]
                        nc.tensor.matmul(
                            psm[:, c0:BLK], ones_sb[:], ap,
                            start=(kt == 0), stop=(kt == nk - 1))

                    # Scores land pairwise in one 2-bank PSUM tile so a
                    # single ACT exp covers N=1024 (halves ACT overhead,
                    # the attention-phase critical cadence). The sums/PV
                    # matmuls trail by 1/2 pairs so PE waits stay
                    # pre-satisfied, and the normalization chain overlaps
                    # the next q-block.
                    for j in range(np_):
                        pa = ps_att.tile([128, 2, BLK], f32, tag="psatt")
                        p2 = probs_pool.tile([128, 2, BLK], bf16, tag="p")
                        if j == np_ - 1:
                            # tail diagonal pair (dj=2,3): only q >= dj*128
                            # can be unmasked, so scores/exp/mask/sums/PV all
                            # run on the [c0:512] subrange; the masked rest is
                            # never written or read (kt=0 writes po/psm
                            # full-width, so accumulation stays sound).
                            for half in range(2):
                                kt = 2 * j + half
                                c0 = (2 + half) * 128
                                nc.tensor.matmul(
                                    pa[:, half, c0:BLK],
                                    kh[:, bass.ts(kt, 128)],
                                    qh[:, qb * BLK + c0:(qb + 1) * BLK],
                                    start=True, stop=True)
                            for half in range(2):
                                kt = 2 * j + half
                                c0 = (2 + half) * 128
                                nc.scalar.activation(
                                    p2[:, half, c0:BLK], pa[:, half, c0:BLK],
                                    mybir.ActivationFunctionType.Exp,
                                    scale=SCALE)
                                nc.vector.tensor_mul(
                                    p2[:, half, c0:BLK], p2[:, half, c0:BLK],
                                    dmask_sb[:, 2 + half, c0:BLK])
                                ktile.append((p2[:, half, c0:BLK], c0))
                        else:
                            # the dj=1 half keeps only q >= 128: shrink its
                            # scores/mask/sums/PV to [128:512] too. The exp
                            # stays full-pair (splitting costs more fixed
                            # overhead than the area saves); it reads stale
                            # PSUM in the unwritten region, whose exp lands
                            # in a probs region nothing ever reads.
                            for half in range(2):
                                kt = 2 * j + half
                                c0 = 128 if kt - 4 * qb == 1 else 0
                                nc.tensor.matmul(
                                    pa[:, half, c0:BLK],
                                    kh[:, bass.ts(kt, 128)],
                                    qh[:, qb * BLK + c0:(qb + 1) * BLK],
                                    start=True, stop=True)
                            nc.scalar.activation(
                                p2[:, :, :], pa[:, :, :],
                                mybir.ActivationFunctionType.Exp, scale=SCALE)
                            for half in range(2):
                                kt = 2 * j + half
                                dj = kt - 4 * qb
                                c0 = 128 if dj == 1 else 0
                                if dj >= 0:
                                    nc.vector.tensor_mul(
                                        p2[:, half, c0:BLK],
                                        p2[:, half, c0:BLK],
                                        dmask_sb[:, dj, c0:BLK])
                                ktile.append((p2[:, half, c0:BLK], c0))
                        if j >= 1:
                            ones_mm(2 * j - 2)
                            ones_mm(2 * j - 1)
                        if j >= 2:
                            pv(2 * j - 4)
                            pv(2 * j - 3)
                    ones_mm(nk - 2)
                    ones_mm(nk - 1)
                    for kt in range(nk - 4, nk):
                        pv(kt)

                    recip = norm_pool.tile([1, BLK], f32, tag="recip")
                    nc.vector.reciprocal_approx_fast(recip[:], psm[:])
                    bcast = norm_pool.tile([128, BLK], f32, tag="bcast")
                    nc.gpsimd.partition_broadcast(bcast[:], recip[:])
                    nc.vector.tensor_mul(
                        att_b[:, h, bass.ts(qb, BLK)], po[:], bcast[:])

            # ---- output projection (partial over this core's heads) ----
            # pairs of 512-col blocks share one 2-bank PSUM tile so the
            # PSUM->SBUF drain is one wide copy per pair, alternating
            # DVE/ACT to keep the drain faster than PE production.
            for tt in range(S // 128):
                last = (b == B - 1 and tt == S // 128 - 1)
                for cbp in range(D // (2 * BLK)):
                    psp = ps_att.tile([128, 2, BLK], f32, tag="psatt")
                    for half in range(2):
                        cb = 2 * cbp + half
                        for f in range(HPC):
                            nc.tensor.matmul(
                                psp[:, half, :], att_b[:, f, bass.ts(tt, 128)],
                                wp_sb[:, f, bass.ts(cb, BLK)],
                                start=(f == 0), stop=(f == HPC - 1))
                    if last:
                        # drain the final tile with parallel narrow copies +
                        # DMAs so the kernel tail is as short as possible
                        for half in range(2):
                            cb = 2 * cbp + half
                            obh = out_pool.tile([128, BLK], f32, tag="obh")
                            if half == 0:
                                nc.vector.tensor_copy(obh[:], psp[:, half, :])
                            else:
                                nc.scalar.copy(obh[:], psp[:, half, :])
                            nc.sync.dma_start(
                                out[bass.ts(b * (S // 128) + tt, 128),
                                    bass.ts(cb, BLK)], obh[:])
                        continue
                    ob = out_pool.tile([128, 2 * BLK], f32, tag="ob")
                    if cbp % 2 == 0:
                        nc.vector.tensor_copy(ob[:], psp[:, :, :])
                    else:
                        nc.scalar.copy(ob[:], psp[:, :, :])
                    nc.sync.dma_start(
                        out[bass.ts(b * (S // 128) + tt, 128),
                            bass.ts(cbp, 2 * BLK)], ob[:])

    nc.compile()
    _cached["nc"] = nc
    return nc


def _prep_inputs(x, sin, cos, Wqkv, Wproj):
    """Host-side shard prep: slice per-core weights, transpose/cast."""
    xt = np.ascontiguousarray(
        x.transpose(0, 2, 1)).astype(ml_dtypes.bfloat16)  # [B, D, S]

    sinT = np.ascontiguousarray(sin[0, 0].T).astype(np.float32)   # [HD, S]
    sinT[:64] = -sinT[:64]
    sints = sinT.astype(ml_dtypes.bfloat16)
    cosT = np.ascontiguousarray(cos[0, 0].T).astype(ml_dtypes.bfloat16)

    r = np.arange(128)[:, None]
    c = np.arange(BLK)[None, :]
    dmask = np.stack(
        [(c >= dj * 128 + r) for dj in range(4)]
    ).astype(ml_dtypes.bfloat16)  # [4, 128, BLK]

    in_maps = []
    for core in range(NCORES):
        h0 = core * HPC
        qcols = Wqkv[:, h0 * HD:(h0 + HPC) * HD]
        kcols = Wqkv[:, D + h0 * HD:D + (h0 + HPC) * HD]
        vcols = Wqkv[:, 2 * D + h0 * HD:2 * D + (h0 + HPC) * HD]
        wqk_c = np.concatenate([qcols, kcols], axis=1).astype(ml_dtypes.bfloat16)
        wv_c = np.ascontiguousarray(vcols).astype(ml_dtypes.bfloat16)
        wp_c = np.ascontiguousarray(
            Wproj[h0 * HD:(h0 + HPC) * HD, :]).astype(ml_dtypes.bfloat16)
        in_maps.append({
            "xt": xt, "wqk": wqk_c, "wv": wv_c, "wp": wp_c,
            "cost": cosT, "sints": sints, "dmask": dmask,
        })
    return in_maps


LAST_RESULT = None


def kernel(x, mask, sin, cos, Wqkv, Wproj):
    global LAST_RESULT
    x = np.asarray(x, dtype=np.float32)
    sin = np.asarray(sin, dtype=np.float32)
    cos = np.asarray(cos, dtype=np.float32)
    Wqkv = np.asarray(Wqkv, dtype=np.float32)
    Wproj = np.asarray(Wproj, dtype=np.float32)
    # mask is the deterministic causal tril from setup_inputs();
    # causality is implemented structurally in the kernel.

    nc = _build()
    in_maps = _prep_inputs(x, sin, cos, Wqkv, Wproj)
    res = run_bass_kernel_spmd(nc, in_maps, core_ids=list(range(NCORES)))
    LAST_RESULT = res

    acc = res.results[0]["out"].astype(np.float32)
    for cres in res.results[1:]:
        acc = acc + cres["out"]
    return acc.reshape(B, S, D)


# revision 33
# speedup vs baseline: 1.1731x; 1.1731x over previous
"""Multi-head attention (RoPE + causal) on 8 TRN2 NeuronCores.

Sharding: tensor-parallel over heads. Core c owns heads {2c, 2c+1} for
both batches: Wqkv column-sharded by head, Wproj row-sharded; each core
produces a partial [B*S, D] output which the host sums.

Per-core pipeline (all matmuls bf16, fp32 PSUM accumulation):
  - x is pre-transposed/cast on host to xT[b] = [D, S] bf16 so the
    contraction dim (d) lands on SBUF partitions.
  - Q^T/K^T computed head-major ([hd, tok]); RoPE applied in that
    layout via partition-shifted DVE ops with host-prepared
    transposed (and sign-folded) sin/cos tables.
  - V computed token-major ([tok, hd]) from the same xT tiles.
  - Scores computed TRANSPOSED: S^T[k, q] = K_tile @ Q^T so softmax'a
    exp runs on ACT straight out of PSUM, probs stay [k, q] = P^T,
    which is exactly the moving operand PV needs -> no on-chip
    transposes anywhere. Row sums via a ones-vector matmul; causal
    handled structurally (upper tiles never computed) + 4 static
    diagonal masks; no max-subtraction (scores are O(1) by
    construction so exp cannot overflow).
  - att^T[hd, q] accumulates in PSUM, normalized by 1/sum broadcast
    (GPSIMD partition_broadcast), and is directly the stationary
    operand for the output projection.
"""

import os
import sys

for _p in ("/opt/trn_rl_repo", "/root/.axon_site/_ro/trn_rl_repo"):
    if os.path.isdir(_p) and _p not in sys.path:
        sys.path.append(_p)

import numpy as np
import ml_dtypes

import concourse.bass as bass
import concourse.mybir as mybir
import concourse.tile as tile
from concourse import bacc
from concourse.bass_utils import run_bass_kernel_spmd

B, S, D, H, HD = 2, 2048, 2048, 16, 128
NCORES = 8
HPC = H // NCORES            # heads per core = 2
QKC = 2 * HPC * HD           # q+k cols per core = 512
VC = HPC * HD                # v cols per core = 256
DC = D // 128                # d chunks = 16
TB = 4                       # token blocks of 512
BLK = S // TB                # 512
SCALE = 1.0 / float(np.sqrt(HD))

bf16 = mybir.dt.bfloat16
f32 = mybir.dt.float32

_cached = {}


def _build():
    if "nc" in _cached:
        return _cached["nc"]
    from contextlib import ExitStack

    nc = bacc.Bacc("TRN2", target_bir_lowering=False, debug=False,
                   num_devices=NCORES)

    xt = nc.dram_tensor("xt", [B, D, S], bf16, kind="ExternalInput")
    wqk = nc.dram_tensor("wqk", [D, QKC], bf16, kind="ExternalInput")
    wv = nc.dram_tensor("wv", [D, VC], bf16, kind="ExternalInput")
    wp = nc.dram_tensor("wp", [VC, D], bf16, kind="ExternalInput")
    cost = nc.dram_tensor("cost", [HD, S], bf16, kind="ExternalInput")
    sints = nc.dram_tensor("sints", [HD, S], bf16, kind="ExternalInput")
    dmask = nc.dram_tensor("dmask", [4, 128, BLK], bf16, kind="ExternalInput")
    out = nc.dram_tensor("out", [B * S, D], f32, kind="ExternalOutput")

    with tile.TileContext(nc) as tc, ExitStack() as ctx:
        cpool = ctx.enter_context(tc.tile_pool(name="consts", bufs=1))
        xt_pool = ctx.enter_context(tc.tile_pool(name="xt", bufs=2))
        qk_pool = ctx.enter_context(tc.tile_pool(name="qk", bufs=2))
        v_pool = ctx.enter_context(tc.tile_pool(name="v", bufs=2))
        att_pool = ctx.enter_context(tc.tile_pool(name="att", bufs=2))
        rope_pool = ctx.enter_context(tc.tile_pool(name="rope", bufs=2))
        probs_pool = ctx.enter_context(tc.tile_pool(name="probs", bufs=6))
        norm_pool = ctx.enter_context(tc.tile_pool(name="norm", bufs=2))
        out_pool = ctx.enter_context(tc.tile_pool(name="out", bufs=4))
        # 8 PSUM banks total: work 2x1 (QK/V accum in qkv phase, PV
        # accumulators in attention - phase-disjoint), 2-bank pairs 2x2
        # (attention scores + proj output), sum 2x1. Double-buffered
        # everywhere so consecutive q-blocks pipeline with no PE stall.
        ps_work = ctx.enter_context(tc.tile_pool(name="pswork", bufs=3, space="PSUM"))
        ps_att = ctx.enter_context(tc.tile_pool(name="psatt", bufs=2, space="PSUM"))
        ps_sum = ctx.enter_context(tc.tile_pool(name="pssum", bufs=1, space="PSUM"))

        # DMA issue order matters for startup: wqk + first xt block feed the
        # first matmul group; everything else can land later.
        wqk_sb = cpool.tile([128, DC, QKC], bf16)
        wqk_r = wqk.rearrange("(c p) n -> p c n", p=128)
        nc.sync.dma_start(wqk_sb[:, 0:DC // 2, :], wqk_r[:, 0:DC // 2, :])
        xt0 = xt_pool.tile([128, DC, BLK], bf16, tag="xtb")
        xt0_r = xt[0].rearrange("(c p) s -> p c s", p=128)[:, :, 0:BLK]
        nc.sync.dma_start(xt0[:, 0:DC // 2, :], xt0_r[:, 0:DC // 2, :])
        nc.sync.dma_start(wqk_sb[:, DC // 2:DC, :], wqk_r[:, DC // 2:DC, :])
        nc.sync.dma_start(xt0[:, DC // 2:DC, :], xt0_r[:, DC // 2:DC, :])
        cost_sb = cpool.tile([HD, S], bf16)
        nc.sync.dma_start(cost_sb[:], cost[:, :])
        sints_sb = cpool.tile([HD, S], bf16)
        nc.sync.dma_start(sints_sb[:], sints[:, :])
        # prefetch the second token block ahead of the late-needed consts
        xt1 = xt_pool.tile([128, DC, BLK], bf16, tag="xtb")
        nc.sync.dma_start(
            xt1[:], xt[0].rearrange("(c p) s -> p c s", p=128)[:, :, BLK:2 * BLK])
        wv_sb = cpool.tile([128, DC, VC], bf16)
        nc.sync.dma_start(wv_sb[:], wv.rearrange("(c p) n -> p c n", p=128))
        dmask_sb = cpool.tile([128, 4, BLK], bf16)
        nc.sync.dma_start(dmask_sb[:], dmask.rearrange("d p n -> p d n"))
        wp_sb = cpool.tile([128, HPC, D], bf16)
        nc.sync.dma_start(wp_sb[:], wp.rearrange("(c p) n -> p c n", p=128))
        ones_sb = cpool.tile([128, 1], bf16)
        nc.vector.memset(ones_sb[:], 1.0)

        # PE warm-up during the initial DMA window: keeps the HAM activity
        # monitor busy so the first real matmuls run at full clock.
        warm_sb = cpool.tile([128, BLK], bf16)
        nc.vector.memset(warm_sb[:], 1.0)
        warm_ps = ps_work.tile([128, BLK], f32, tag="work")
        for _ in range(40):
            nc.tensor.matmul(warm_ps[:], warm_sb[:, 0:128], warm_sb[:],
                             start=True, stop=True)
        nc.vector.tensor_copy(warm_sb[:], warm_ps[:])

        for b in range(B):
            qk_b = qk_pool.tile([128, 2 * HPC, S], bf16, tag="qkb")
            v_b = v_pool.tile([128, 4 * TB, VC], bf16, tag="vb")
            att_b = att_pool.tile([128, HPC, S], bf16, tag="attb")

            # ---- QKV projection for batch b ----
            for tb in range(TB):
                if b == 0 and tb == 0:
                    xt_blk = xt0
                elif b == 0 and tb == 1:
                    xt_blk = xt1
                elif b == 1 and tb == 0:
                    xt_blk = xt_next
                else:
                    xt_blk = xt_pool.tile([128, DC, BLK], bf16, tag="xtb")
                    nc.sync.dma_start(
                        xt_blk[:],
                        xt[b].rearrange("(c p) s -> p c s", p=128)[
                            :, :, bass.ts(tb, BLK)],
                    )
                # Q^T / K^T (head-major) + RoPE
                for ct in range(2 * HPC):
                    ps = ps_work.tile([128, BLK], f32, tag="work")
                    for c in range(DC):
                        nc.tensor.matmul(
                            ps[:], wqk_sb[:, c, bass.ts(ct, 128)],
                            xt_blk[:, c, :],
                            start=(c == 0), stop=(c == DC - 1))
                    cs = cost_sb[:, bass.ts(tb, BLK)]
                    sn = sints_sb[:, bass.ts(tb, BLK)]
                    t1 = rope_pool.tile([128, BLK], f32, tag="t1")
                    t2 = rope_pool.tile([128, BLK], f32, tag="t2")
                    # t1 = rot_half(ps) * sin (sign folded into table)
                    nc.vector.tensor_mul(t1[0:64, :], ps[64:128, :], sn[0:64, :])
                    nc.vector.tensor_mul(t1[64:128, :], ps[0:64, :], sn[64:128, :])
                    nc.vector.tensor_mul(t2[:], ps[:], cs)
                    nc.vector.tensor_add(
                        qk_b[:, ct, bass.ts(tb, BLK)], t1[:], t2[:])
                # V (token-major)
                for ts4 in range(4):
                    psv = ps_work.tile([128, VC], f32, tag="work")
                    for c in range(DC):
                        nc.tensor.matmul(
                            psv[:], xt_blk[:, c, bass.ts(ts4, 128)],
                            wv_sb[:, c, :],
                            start=(c == 0), stop=(c == DC - 1))
                    nc.vector.tensor_copy(v_b[:, tb * 4 + ts4, :], psv[:])

            if b == 0:
                # prefetch batch 1's first token block now: the xt slot is
                # free through b0's attention/proj, and issuing the DMA here
                # keeps it ahead of proj0's out-DMAs in the sync queue.
                xt_next = xt_pool.tile([128, DC, BLK], bf16, tag="xtb")
                nc.sync.dma_start(
                    xt_next[:],
                    xt[1].rearrange("(c p) s -> p c s", p=128)[:, :, 0:BLK])

            # ---- causal attention, heads h = 0..HPC-1 ----
            for h in range(HPC):
                qh = qk_b[:, h, :]
                kh = qk_b[:, HPC + h, :]
                for qb in range(TB):
                    nk = 4 * (qb + 1)
                    np_ = nk // 2
                    po = ps_work.tile([128, BLK], f32, tag="work")
                    psm = ps_sum.tile([1, BLK], f32, tag="pssum")
                    ktile = []

                    def pv(kt):
                        ap, c0 = ktile[kt]
                        nc.tensor.matmul(
                            po[:, c0:BLK], v_b[:, kt, bass.ts(h, HD)], ap,
                            start=(kt == 0), stop=(kt == nk - 1))

                    def ones_mm(kt):
                        ap, c0 = ktile[kt]
                        nc.tensor.matmul(
                            psm[:, c0:BLK], ones_sb[:], ap,
                            start=(kt == 0), stop=(kt == nk - 1))

                    # Scores land pairwise in one 2-bank PSUM tile so a
                    # single ACT exp covers N=1024 (halves ACT overhead,
                    # the attention-phase critical cadence). The sums/PV
                    # matmuls trail by 1/2 pairs so PE waits stay
                    # pre-satisfied, and the normalization chain overlaps
                    # the next q-block.
                    for j in range(np_):
                        pa = ps_att.tile([128, 2, BLK], f32, tag="psatt")
                        p2 = probs_pool.tile([128, 2, BLK], bf16, tag="p")
                        if j == np_ - 1:
                            # tail diagonal pair (dj=2,3): only q >= dj*128
                            # can be unmasked, so scores/exp/mask/sums/PV all
                            # run on the [c0:512] subrange; the masked rest is
                            # never written or read (kt=0 writes po/psm
                            # full-width, so accumulation stays sound).
                            for half in range(2):
                                kt = 2 * j + half
                                c0 = (2 + half) * 128
                                nc.tensor.matmul(
                                    pa[:, half, c0:BLK],
                                    kh[:, bass.ts(kt, 128)],
                                    qh[:, qb * BLK + c0:(qb + 1) * BLK],
                                    start=True, stop=True)
                            for half in range(2):
                                kt = 2 * j + half
                                c0 = (2 + half) * 128
                                nc.scalar.activation(
                                    p2[:, half, c0:BLK], pa[:, half, c0:BLK],
                                    mybir.ActivationFunctionType.Exp,
                                    scale=SCALE)
                                nc.vector.tensor_mul(
                                    p2[:, half, c0:BLK], p2[:, half, c0:BLK],
                                    dmask_sb[:, 2 + half, c0:BLK])
                                ktile.append((p2[:, half, c0:BLK], c0))
                        else:
                            for half in range(2):
                                kt = 2 * j + half
                                nc.tensor.matmul(
                                    pa[:, half, :], kh[:, bass.ts(kt, 128)],
                                    qh[:, bass.ts(qb, BLK)],
                                    start=True, stop=True)
                            nc.scalar.activation(
                                p2[:, :, :], pa[:, :, :],
                                mybir.ActivationFunctionType.Exp, scale=SCALE)
                            for half in range(2):
                                kt = 2 * j + half
                                dj = kt - 4 * qb
                                if dj >= 0:
                                    nc.vector.tensor_mul(
                                        p2[:, half, :], p2[:, half, :],
                                        dmask_sb[:, dj, :])
                                ktile.append((p2[:, half, :], 0))
                        if j >= 1:
                            ones_mm(2 * j - 2)
                            ones_mm(2 * j - 1)
                        if j >= 2:
                            pv(2 * j - 4)
                            pv(2 * j - 3)
                    ones_mm(nk - 2)
                    ones_mm(nk - 1)
                    for kt in range(nk - 4, nk):
                        pv(kt)

                    recip = norm_pool.tile([1, BLK], f32, tag="recip")
                    nc.vector.reciprocal_approx_fast(recip[:], psm[:])
                    bcast = norm_pool.tile([128, BLK], f32, tag="bcast")
                    nc.gpsimd.partition_broadcast(bcast[:], recip[:])
                    nc.vector.tensor_mul(
                        att_b[:, h, bass.ts(qb, BLK)], po[:], bcast[:])

            # ---- output projection (partial over this core's heads) ----
            # pairs of 512-col blocks share one 2-bank PSUM tile so the
            # PSUM->SBUF drain is one wide copy per pair, alternating
            # DVE/ACT to keep the drain faster than PE production.
            for tt in range(S // 128):
                last = (b == B - 1 and tt == S // 128 - 1)
                for cbp in range(D // (2 * BLK)):
                    psp = ps_att.tile([128, 2, BLK], f32, tag="psatt")
                    for half in range(2):
                        cb = 2 * cbp + half
                        for f in range(HPC):
                            nc.tensor.matmul(
                                psp[:, half, :], att_b[:, f, bass.ts(tt, 128)],
                                wp_sb[:, f, bass.ts(cb, BLK)],
                                start=(f == 0), stop=(f == HPC - 1))
                    if last:
                        # drain the final tile with parallel narrow copies +
                        # DMAs so the kernel tail is as short as possible
                        for half in range(2):
                            cb = 2 * cbp + half
                            obh = out_pool.tile([128, BLK], f32, tag="obh")
                            if half == 0:
                                nc.vector.tensor_copy(obh[:], psp[:, half, :])
                            else:
                                nc.scalar.copy(obh[:], psp[:, half, :])
                            nc.sync.dma_start(
                                out[bass.ts(b * (S // 128) + tt, 128),
                                    bass.ts(cb, BLK)], obh[:])
                        continue
                    ob = out_pool.tile([128, 2 * BLK], f32, tag="ob")
                    if cbp % 2 == 0:
                        nc.vector.tensor_copy(ob[:], psp[:, :, :])
                    else:
                        nc.scalar.copy(ob[:], psp[:, :, :])
                    nc.sync.dma_start(
                        out[bass.ts(b * (S // 128) + tt, 128),
                            bass.ts(cbp, 2 * BLK)], ob[:])

    nc.compile()
    _cached["nc"] = nc
    return nc


def _prep_inputs(x, sin, cos, Wqkv, Wproj):
    """Host-side shard prep: slice per-core weights, transpose/cast."""
    xt = np.ascontiguousarray(
        x.transpose(0, 2, 1)).astype(ml_dtypes.bfloat16)  # [B, D, S]

    sinT = np.ascontiguousarray(sin[0, 0].T).astype(np.float32)   # [HD, S]
    sinT[:64] = -sinT[:64]
    sints = sinT.astype(ml_dtypes.bfloat16)
    cosT = np.ascontiguousarray(cos[0, 0].T).astype(ml_dtypes.bfloat16)

    r = np.arange(128)[:, None]
    c = np.arange(BLK)[None, :]
    dmask = np.stack(
        [(c >= dj * 128 + r) for dj in range(4)]
    ).astype(ml_dtypes.bfloat16)  # [4, 128, BLK]

    in_maps = []
    for core in range(NCORES):
        h0 = core * HPC
        qcols = Wqkv[:, h0 * HD:(h0 + HPC) * HD]
        kcols = Wqkv[:, D + h0 * HD:D + (h0 + HPC) * HD]
        vcols = Wqkv[:, 2 * D + h0 * HD:2 * D + (h0 + HPC) * HD]
        wqk_c = np.concatenate([qcols, kcols], axis=1).astype(ml_dtypes.bfloat16)
        wv_c = np.ascontiguousarray(vcols).astype(ml_dtypes.bfloat16)
        wp_c = np.ascontiguousarray(
            Wproj[h0 * HD:(h0 + HPC) * HD, :]).astype(ml_dtypes.bfloat16)
        in_maps.append({
            "xt": xt, "wqk": wqk_c, "wv": wv_c, "wp": wp_c,
            "cost": cosT, "sints": sints, "dmask": dmask,
        })
    return in_maps


LAST_RESULT = None


def kernel(x, mask, sin, cos, Wqkv, Wproj):
    global LAST_RESULT
    x = np.asarray(x, dtype=np.float32)
    sin = np.asarray(sin, dtype=np.float32)
    cos = np.asarray(cos, dtype=np.float32)
    Wqkv = np.asarray(Wqkv, dtype=np.float32)
    Wproj = np.asarray(Wproj, dtype=np.float32)
    # mask is the deterministic causal tril from setup_inputs();
    # causality is implemented structurally in the kernel.

    nc = _build()
    in_maps = _prep_inputs(x, sin, cos, Wqkv, Wproj)
    res = run_bass_kernel_spmd(nc, in_maps, core_ids=list(range(NCORES)))
    LAST_RESULT = res

    acc = res.results[0]["out"].astype(np.float32)
    for cres in res.results[1:]:
        acc = acc + cres["out"]
    return acc.reshape(B, S, D)
